# revision 23
# baseline (speedup 1.0000x reference)
"""MatchingNetworks forward as a Trainium2 Bass/Tile kernel, SPMD over 8 cores.

Math (per the reference):
    qe = Xq @ W + b            [Q, 64]
    se = Xs @ W + b            [S, 64]
    sims = l2n(qe) @ l2n(se).T [Q, S]
    attn = softmax(sims, axis=1)
    out  = attn @ one_hot(labels, 20)

Sharding: data-parallel over Q (512 queries per core); support set, weights
and bias replicated. Each core is fully independent (no collectives).

Device layout: the contraction dim D_in=21168 must live on SBUF partitions
for the TensorE matmuls, so the host pre-packs W|Xs^T|Xq^T K-tile-interleaved
into ONE combined [128, T*676] bf16 buffer (per K-tile t: 64 cols of W,
100 cols of Xs^T, 512 cols of Xq^T; partition p holds contraction row
t*128+p). One buffer means one DMA per chunk of K-tiles, which gives
fully-contiguous per-partition reads. bf16 halves the HBM traffic (the
kernel is DMA-bound); PSUM accumulation is fp32 and everything after the
embedding GEMM is fp32.

The embedding GEMMs compute emb^T = W.T @ X^T ([64, n] PSUM accumulators
over 166 K-tiles). The bias is added during the PSUM->SBUF activations
(bias is per-partition in the emb^T orientation). Norms use ones-vector
matmuls; softmax runs in [q, s] orientation on ACT/DVE; the label
segment-sum is a one_hot matmul in transposed orientation with small PE
transposes.

Perf notes:
  - The query GEMM runs on PE array columns 0-63 and the support GEMM on
    columns 64-127 (tile_position col-tiling): the two matmuls per K-tile
    execute concurrently in the array and share one PSUM bank, so the
    support stream (100 cols) hides entirely under the query stream (512).
  - The tail is phased (all SQRTs, then all EXPs) so the ACT engine's
    function table switches once instead of per q-subtile (each
    ACT_TABLE_LOAD costs 1.28us on the critical path).
  - Softmax is unstabilized (no max subtraction): cosines are in [-1, 1]
    so exp cannot overflow, and the 1/||qe|| scale is fused into the Exp's
    per-partition scale operand.
  - This walrus build rejects any instruction carrying more than one sync
    wait; legalize_single_wait() post-processes the scheduled BIR, hoisting
    extra waits onto same-engine NoOps inserted before the instruction
    (identical semantics, since each engine executes its queue in order).
"""

import sys

sys.path.insert(0, "/opt/trn_rl_repo")

import numpy as np
import ml_dtypes

import concourse.bass as bass
import concourse.mybir as mybir
import concourse.tile as tile
from concourse.bass_utils import run_bass_kernel_spmd

# Problem constants (hardcoded per the grading contract).
S = 100
Q = 4096
D_IN = 21168
D = 64
NWAY = 20
NCORES = 8
QC = Q // NCORES  # 512 queries per core
KP = 128
T = (D_IN + KP - 1) // KP  # 166 K-tiles
KPAD = T * KP  # 21248
CH = 8  # K-tiles per DMA chunk
TW = D + QC  # 576 columns per K-tile in the main (query) stream
TWS = D + S  # 164 columns per K-tile in the per-core support stream
NBUF = 6  # chunk double-buffering depth

F32 = mybir.dt.float32
BF16 = mybir.dt.bfloat16
BF16_NP = ml_dtypes.bfloat16


def _chunks(total, ch):
    t0 = 0
    while t0 < total:
        c = min(ch, total - t0)
        yield t0, c
        t0 += c


def build_bass(t_tiles=T):
    """Build the per-core Bass program. t_tiles shrinks the K extent for sim."""
    nts = (t_tiles + NCORES - 1) // NCORES  # support K-tiles per core
    nc = bass.Bass()
    data = nc.dram_tensor("data", [KP, t_tiles * TW], BF16, kind="ExternalInput")
    sup = nc.dram_tensor("sup", [KP, nts * TWS], BF16, kind="ExternalInput")
    bias = nc.dram_tensor("bias", [D, 1], F32, kind="ExternalInput")
    onehot = nc.dram_tensor("onehot", [S, NWAY], F32, kind="ExternalInput")
    identity = nc.dram_tensor("identity", [KP, KP], F32, kind="ExternalInput")
    ones = nc.dram_tensor("ones", [D, 1], F32, kind="ExternalInput")
    out = nc.dram_tensor("out", [QC, NWAY], F32, kind="ExternalOutput")

    AF = mybir.ActivationFunctionType
    AX = mybir.AxisListType

    with tile.TileContext(nc) as tc:
        with (
            tc.tile_pool(name="const", bufs=1) as const,
            tc.tile_pool(name="stream", bufs=NBUF) as stream,
            tc.tile_pool(name="sb", bufs=4) as sb,
            tc.tile_pool(name="dram", bufs=1, space="DRAM") as dram,
            tc.tile_pool(name="ps_q", bufs=1, space="PSUM") as ps_q,
            tc.tile_pool(name="ps_s", bufs=1, space="PSUM") as ps_s,
            tc.tile_pool(name="ps_t", bufs=6, space="PSUM") as ps_t,
        ):
            sup_sb = const.tile([KP, nts * TWS], BF16)
            nc.sync.dma_start(sup_sb[:], sup[:])
            ident = const.tile([128, 128], F32)
            nc.sync.dma_start(ident[:], identity[:])
            bias_col = const.tile([D, 1], F32)
            nc.sync.dma_start(bias_col[:], bias[:])
            oh_sb = const.tile([S, NWAY], F32)
            nc.sync.dma_start(oh_sb[:], onehot[:])
            ones_col = const.tile([D, 1], F32)
            nc.sync.dma_start(ones_col[:], ones[:])

            # One shared PSUM bank for both emb^T accumulators: the query GEMM
            # runs on array columns 0-63 (out partitions 0-63), the support
            # GEMM on columns 64-127 (out partitions 64-127) via tile_position.
            # The support contraction is K-SHARDED across the 8 cores: each
            # core contracts only its nts private K-tiles (from the small
            # `sup` stream) and the [64, 100] partials are AllReduced mid-loop,
            # fully overlapped with the remaining query K-tiles.
            acc = ps_q.tile([KP, QC], F32)
            embq_ps = acc[:D, :]
            # own bank: the DVE partial-copy must not serialize against the
            # query accumulation still writing the other bank
            acc_s = ps_s.tile([KP, S], F32)
            sembT_ps = acc_s[D : 2 * D, :]

            # Main K loop: one DMA per chunk of K-tiles for the query GEMM;
            # the first nts iterations also run this core's support matmuls.
            for ci, (t0, csz) in enumerate(_chunks(t_tiles, CH)):
                chunk = stream.tile([KP, csz * TW], BF16, tag="chunk")
                nc.sync.dma_start(chunk[:], data[:, t0 * TW : (t0 + csz) * TW])
                for i in range(csz):
                    t = t0 + i
                    base = i * TW
                    wt = chunk[:, base : base + D]
                    xqt = chunk[:, base + D : base + TW]
                    nc.tensor.matmul(
                        embq_ps,
                        lhsT=wt,
                        rhs=xqt,
                        start=(t == 0),
                        stop=(t == t_tiles - 1),
                        tile_position=(0, 0),
                        skip_group_check=True,
                    )
                    if t < nts:
                        sbase = t * TWS
                        wst = sup_sb[:, sbase : sbase + D]
                        xst = sup_sb[:, sbase + D : sbase + TWS]
                        nc.tensor.matmul(
                            sembT_ps,
                            lhsT=wst,
                            rhs=xst,
                            start=(t == 0),
                            stop=(t == nts - 1),
                            tile_position=(0, D),
                            skip_group_check=True,
                        )

            # ---- support reduce + normalize (overlaps the rest of the loop) --
            part_sb = sb.tile([D, S], F32, bufs=1)
            nc.vector.tensor_copy(part_sb[:], sembT_ps)
            cc_in = dram.tile([D, S], F32)
            nc.sync.dma_start(cc_in[:], part_sb[:])
            cc_out = dram.tile([D, S], F32)
            nc.gpsimd.collective_compute(
                "AllReduce",
                mybir.AluOpType.add,
                replica_groups=[list(range(NCORES))],
                ins=[cc_in[:].opt()],
                outs=[cc_out[:].opt()],
            )
            red_sb = sb.tile([D, S], F32, bufs=1)
            nc.sync.dma_start(red_sb[:], cc_out[:])

            sT = sb.tile([D, S], F32, bufs=1)
            nc.scalar.activation(sT[:], red_sb[:], AF.Identity, bias=bias_col[:])
            ssq = sb.tile([D, S], F32, bufs=1)
            nc.scalar.activation(ssq[:], red_sb[:], AF.Square, bias=bias_col[:])
            ns_ps = ps_t.tile([S, 1], F32, tag="tp")
            nc.tensor.matmul(ns_ps[:], lhsT=ssq[:], rhs=ones_col[:], start=True, stop=True)
            nsc = sb.tile([S, 1], F32, bufs=1)
            nc.vector.tensor_copy(nsc[:], ns_ps[:])
            nsi = sb.tile([S, 1], F32, bufs=1)
            nc.vector.reciprocal(nsi[:], nsc[:])
            cs = sb.tile([S, 1], F32, bufs=1)
            nc.scalar.sqrt(cs[:], nsi[:])  # 1/||se||
            semb_ps = ps_t.tile([S, D], F32, tag="tp")
            nc.tensor.transpose(semb_ps[:], sT[:], ident[:D, :D])
            sn = sb.tile([S, D], F32, bufs=1)
            nc.scalar.mul(sn[:], semb_ps[:], cs[:])
            snT_ps = ps_t.tile([D, S], F32, tag="tp")
            nc.tensor.transpose(snT_ps[:], sn[:], ident[:S, :S])
            snT = sb.tile([D, S], F32, bufs=1)
            nc.vector.tensor_copy(snT[:], snT_ps[:])

            # ---- query tail, phased so the ACT function-table only switches
            # once (SQRT bank -> EXP bank); alternating per-j cost 1.28us per
            # ACT_TABLE_LOAD on the critical path. ----
            qT = sb.tile([D, QC], F32, bufs=1)
            nc.scalar.activation(qT[:], embq_ps, AF.Identity, bias=bias_col[:])
            sqT = sb.tile([D, QC], F32, bufs=1)
            nc.scalar.activation(sqT[:], embq_ps, AF.Square, bias=bias_col[:])

            NJ = QC // 128
            aq, sims_ps, etile, denom, rden = [], [], [], [], []
            # phase B: query norms (all SQRTs together)
            for j in range(NJ):
                qs = slice(j * 128, (j + 1) * 128)
                nq_ps = ps_t.tile([128, 1], F32, tag="tp")
                nc.tensor.matmul(
                    nq_ps[:], lhsT=sqT[:, qs], rhs=ones_col[:], start=True, stop=True
                )
                nqi = sb.tile([128, 1], F32, tag="nqi")
                nc.vector.reciprocal(nqi[:], nq_ps[:])
                a = sb.tile([128, 1], F32, tag="aq")
                nc.scalar.sqrt(a[:], nqi[:])  # 1/||qe||
                aq.append(a)
            # phase C: sims + exp (all EXPs together; one table switch)
            for j in range(NJ):
                qs = slice(j * 128, (j + 1) * 128)
                sp = ps_t.tile([128, S], F32, tag="tp")
                nc.tensor.matmul(sp[:], lhsT=qT[:, qs], rhs=snT[:], start=True, stop=True)
                sims_ps.append(sp)
                # exp(sims * 1/||qe||) directly: cosines are in [-1, 1] so the
                # unstabilized softmax cannot overflow.
                e = sb.tile([128, S], F32, tag="etile")
                dn = sb.tile([128, 1], F32, tag="denom")
                nc.scalar.activation(e[:], sp[:], AF.Exp, scale=aq[j][:], accum_out=dn[:])
                etile.append(e)
                denom.append(dn)
            # phase D: denominators, transposed one_hot segment-sum, output
            for j in range(NJ):
                r = sb.tile([128, 1], F32, tag="rden")
                nc.vector.reciprocal(r[:], denom[j][:])
                rden.append(r)
            for j in range(NJ):
                qs = slice(j * 128, (j + 1) * 128)
                eT_ps = ps_t.tile([S, 128], F32, tag="tp")
                nc.tensor.transpose(eT_ps[:], etile[j][:], ident[:])
                eT = sb.tile([S, 128], F32, tag="eT")
                nc.vector.tensor_copy(eT[:], eT_ps[:])
                numT_ps = ps_t.tile([NWAY, 128], F32, tag="tp")
                nc.tensor.matmul(
                    numT_ps[:], lhsT=oh_sb[:], rhs=eT[:], start=True, stop=True
                )
                numT = sb.tile([NWAY, 128], F32, tag="numT")
                nc.vector.tensor_copy(numT[:], numT_ps[:])
                num_ps = ps_t.tile([128, NWAY], F32, tag="tp")
                nc.tensor.transpose(num_ps[:], numT[:], ident[:NWAY, :NWAY])
                lg = sb.tile([128, NWAY], F32, tag="lg")
                nc.vector.tensor_scalar_mul(lg[:], num_ps[:], rden[j][:])
                nc.sync.dma_start(out[qs, :], lg[:])
    return nc


def legalize_single_wait(nc):
    """Split multi-wait instructions: this walrus build allows at most ONE
    sync wait per instruction, so hoist extra waits onto same-engine NoOps
    inserted immediately before the instruction (identical semantics: the
    engine executes its queue in order)."""
    import bass_rust

    ctr = 0
    nsplit = 0
    for f in nc.m.functions:
        for bb in f.blocks:
            il = bb.instructions
            i = 0
            while i < len(il):
                ins = il[i]
                si = getattr(ins, "sync_info", None)
                if si is not None and len(si.on_wait) > 1:
                    waits = list(si.on_wait)
                    for w in waits[:-1]:
                        nop = bass_rust.InstNoOp(name=f"W-split-{ctr}")
                        ctr += 1
                        nop.engine = ins.engine
                        nop.sync_info = bass_rust.SyncInfo(on_wait=[w], on_update=[])
                        il.insert(i, nop)
                        i += 1
                    ins.sync_info = bass_rust.SyncInfo(
                        on_wait=[waits[-1]], on_update=list(si.on_update)
                    )
                    nsplit += 1
                i += 1
    # verify the rewrite took (bb.instructions must be a live list)
    remaining = sum(
        1
        for f in nc.m.functions
        for bb in f.blocks
        for ins in bb.instructions
        if getattr(ins, "sync_info", None) is not None
        and len(ins.sync_info.on_wait) > 1
    )
    assert remaining == 0, f"legalize_single_wait: {remaining} multi-wait instrs left"
    return nc


def _wp16(W_, kext):
    Wp = np.zeros((kext, D), dtype=BF16_NP)
    Wp[: W_.shape[0]] = W_.astype(BF16_NP)
    return Wp


def _rows_pack(X, n, t_tiles):
    kext = t_tiles * KP
    Xp = np.zeros((n, kext), dtype=BF16_NP)
    Xp[:, : X.shape[1]] = X.astype(BF16_NP)
    return Xp.reshape(n, t_tiles, KP).transpose(2, 1, 0)  # [128, t, n]


def pack_main(Wp, Xq_core, t_tiles=T):
    """[128, t_tiles*TW] bf16: per K-tile t, cols [0:64)=W rows,
    cols [64:576)=Xq_core^T rows (xq_t[p, j] = Xq[j, t*128+p])."""
    A = np.zeros((KP, t_tiles, TW), dtype=BF16_NP)
    A[:, :, :D] = Wp.reshape(t_tiles, KP, D).transpose(1, 0, 2)
    A[:, :, D:] = _rows_pack(Xq_core, QC, t_tiles)
    return np.ascontiguousarray(A.reshape(KP, t_tiles * TW))


def pack_sup(Wp, Xs_, core, t_tiles=T):
    """Per-core support stream [128, nts*TWS] bf16: this core's K-tile slice
    [core*nts, (core+1)*nts) of W and Xs^T, zero-padded past t_tiles."""
    nts = (t_tiles + NCORES - 1) // NCORES
    kext = t_tiles * KP
    wr = Wp.reshape(t_tiles, KP, D)
    xr = _rows_pack(Xs_, S, t_tiles)  # [128, t, S]
    A = np.zeros((KP, nts, TWS), dtype=BF16_NP)
    for j in range(nts):
        kt = core * nts + j
        if kt < t_tiles:
            A[:, j, :D] = wr[kt]
            A[:, j, D:] = xr[:, kt, :]
    return np.ascontiguousarray(A.reshape(KP, nts * TWS))


def make_in_maps(support_images, support_labels, query_images, backbone_w, backbone_b):
    Xq = np.asarray(query_images, dtype=np.float32)
    Xs = np.asarray(support_images, dtype=np.float32)
    W = np.asarray(backbone_w, dtype=np.float32)
    b = np.asarray(backbone_b, dtype=np.float32).reshape(D, 1)
    labels = np.asarray(support_labels).astype(np.int64).reshape(S)
    onehot = np.zeros((S, NWAY), np.float32)
    onehot[np.arange(S), labels] = 1.0

    Wp = _wp16(W, T * KP)
    common = {
        "bias": b,
        "onehot": onehot,
        "identity": np.eye(KP, dtype=np.float32),
        "ones": np.ones((D, 1), np.float32),
    }
    in_maps = []
    for c in range(NCORES):
        in_maps.append(
            {
                "data": pack_main(Wp, Xq[c * QC : (c + 1) * QC]),
                "sup": pack_sup(Wp, Xs, c),
                **common,
            }
        )
    return in_maps


def run(in_maps, trace=False, **kw):
    nc = build_bass()
    legalize_single_wait(nc)
    return run_bass_kernel_spmd(nc, in_maps, list(range(NCORES)), trace=trace, **kw)


def kernel(
    support_images,
    support_labels,
    query_images,
    n_way,
    k_shot,
    backbone_w,
    backbone_b,
):
    assert int(n_way) == NWAY
    in_maps = make_in_maps(
        support_images, support_labels, query_images, backbone_w, backbone_b
    )
    res = run(in_maps, trace=False)
    return np.concatenate(
        [np.asarray(res.results[c]["out"]) for c in range(NCORES)], axis=0
    )


# revision 25
# speedup vs baseline: 1.2565x; 1.2565x over previous
"""MatchingNetworks forward as a Trainium2 Bass/Tile kernel, SPMD over 8 cores.

Math (per the reference):
    qe = Xq @ W + b            [Q, 64]
    se = Xs @ W + b            [S, 64]
    sims = l2n(qe) @ l2n(se).T [Q, S]
    attn = softmax(sims, axis=1)
    out  = attn @ one_hot(labels, 20)

Sharding: data-parallel over Q (512 queries per core); support set, weights
and bias replicated. Each core is fully independent (no collectives).

Device layout: the contraction dim D_in=21168 must live on SBUF partitions
for the TensorE matmuls, so the host pre-packs W|Xs^T|Xq^T K-tile-interleaved
into ONE combined [128, T*676] bf16 buffer (per K-tile t: 64 cols of W,
100 cols of Xs^T, 512 cols of Xq^T; partition p holds contraction row
t*128+p). One buffer means one DMA per chunk of K-tiles, which gives
fully-contiguous per-partition reads. bf16 halves the HBM traffic (the
kernel is DMA-bound); PSUM accumulation is fp32 and everything after the
embedding GEMM is fp32.

The embedding GEMMs compute emb^T = W.T @ X^T ([64, n] PSUM accumulators
over 166 K-tiles). The bias is added during the PSUM->SBUF activations
(bias is per-partition in the emb^T orientation). Norms use ones-vector
matmuls; softmax runs in [q, s] orientation on ACT/DVE; the label
segment-sum is a one_hot matmul in transposed orientation with small PE
transposes.

Perf notes:
  - The query GEMM runs on PE array columns 0-63 and the support GEMM on
    columns 64-127 (tile_position col-tiling): the two matmuls per K-tile
    execute concurrently in the array and share one PSUM bank, so the
    support stream (100 cols) hides entirely under the query stream (512).
  - The tail is phased (all SQRTs, then all EXPs) so the ACT engine's
    function table switches once instead of per q-subtile (each
    ACT_TABLE_LOAD costs 1.28us on the critical path).
  - Softmax is unstabilized (no max subtraction): cosines are in [-1, 1]
    so exp cannot overflow, and the 1/||qe|| scale is fused into the Exp's
    per-partition scale operand.
  - This walrus build rejects any instruction carrying more than one sync
    wait; legalize_single_wait() post-processes the scheduled BIR, hoisting
    extra waits onto same-engine NoOps inserted before the instruction
    (identical semantics, since each engine executes its queue in order).
"""

import sys

sys.path.insert(0, "/opt/trn_rl_repo")

import numpy as np
import ml_dtypes

import concourse.bass as bass
import concourse.mybir as mybir
import concourse.tile as tile
from concourse.bass_utils import run_bass_kernel_spmd

# Problem constants (hardcoded per the grading contract).
S = 100
Q = 4096
D_IN = 21168
D = 64
NWAY = 20
NCORES = 8
QC = Q // NCORES  # 512 queries per core
KP = 128
T = (D_IN + KP - 1) // KP  # 166 K-tiles
KPAD = T * KP  # 21248
CH = 8  # K-tiles per DMA chunk
TW = D + S + QC  # 676 columns per K-tile in the combined buffer
NBUF = 6  # chunk double-buffering depth

F32 = mybir.dt.float32
BF16 = mybir.dt.bfloat16
BF16_NP = ml_dtypes.bfloat16


def _chunks(total, ch):
    t0 = 0
    while t0 < total:
        c = min(ch, total - t0)
        yield t0, c
        t0 += c


def build_bass(t_tiles=T):
    """Build the per-core Bass program. t_tiles shrinks the K extent for sim."""
    nc = bass.Bass()
    data = nc.dram_tensor("data", [KP, t_tiles * TW], BF16, kind="ExternalInput")
    bias = nc.dram_tensor("bias", [D, 1], F32, kind="ExternalInput")
    onehot = nc.dram_tensor("onehot", [S, NWAY], F32, kind="ExternalInput")
    identity = nc.dram_tensor("identity", [KP, KP], F32, kind="ExternalInput")
    ones = nc.dram_tensor("ones", [D, 1], F32, kind="ExternalInput")
    out = nc.dram_tensor("out", [QC, NWAY], F32, kind="ExternalOutput")

    AF = mybir.ActivationFunctionType
    AX = mybir.AxisListType

    with tile.TileContext(nc) as tc:
        with (
            tc.tile_pool(name="const", bufs=1) as const,
            tc.tile_pool(name="stream", bufs=NBUF) as stream,
            tc.tile_pool(name="sb", bufs=4) as sb,
            tc.tile_pool(name="ps_q", bufs=1, space="PSUM") as ps_q,
            tc.tile_pool(name="ps_t", bufs=6, space="PSUM") as ps_t,
        ):
            # first data chunk goes on the DMA ring before the constants:
            # the PE's first matmul only needs chunk 0
            first_csz = min(CH, t_tiles)
            chunk0 = stream.tile([KP, first_csz * TW], BF16, tag="chunk")
            nc.sync.dma_start(chunk0[:], data[:, : first_csz * TW])

            ident = const.tile([128, 128], F32)
            nc.sync.dma_start(ident[:], identity[:])
            bias_col = const.tile([D, 1], F32)
            nc.sync.dma_start(bias_col[:], bias[:])
            oh_sb = const.tile([S, NWAY], F32)
            nc.sync.dma_start(oh_sb[:], onehot[:])
            ones_col = const.tile([D, 1], F32)
            nc.sync.dma_start(ones_col[:], ones[:])

            # One shared PSUM bank for both emb^T accumulators: the query GEMM
            # runs on array columns 0-63 (out partitions 0-63), the support
            # GEMM on columns 64-127 (out partitions 64-127) via tile_position.
            # The two matmuls per K-tile then overlap inside the PE array and
            # their weight loads hide in the 64-deep reorder window.
            acc = ps_q.tile([KP, QC], F32)
            embq_ps = acc[:D, :]
            sembT_ps = acc[D : 2 * D, :S]

            # Main K loop: one DMA per chunk of K-tiles, accumulate both GEMMs.
            for ci, (t0, csz) in enumerate(_chunks(t_tiles, CH)):
                if ci == 0:
                    chunk = chunk0
                else:
                    chunk = stream.tile([KP, csz * TW], BF16, tag="chunk")
                    nc.sync.dma_start(chunk[:], data[:, t0 * TW : (t0 + csz) * TW])
                for i in range(csz):
                    t = t0 + i
                    base = i * TW
                    wt = chunk[:, base : base + D]
                    xst = chunk[:, base + D : base + D + S]
                    xqt = chunk[:, base + D + S : base + TW]
                    nc.tensor.matmul(
                        embq_ps,
                        lhsT=wt,
                        rhs=xqt,
                        start=(t == 0),
                        stop=(t == t_tiles - 1),
                        tile_position=(0, 0),
                        skip_group_check=True,
                    )
                    nc.tensor.matmul(
                        sembT_ps,
                        lhsT=wt,
                        rhs=xst,
                        start=(t == 0),
                        stop=(t == t_tiles - 1),
                        tile_position=(0, D),
                        skip_group_check=True,
                    )

            # ---- support tail: bias add + normalize the support embeddings ----
            sT = sb.tile([D, S], F32, bufs=1)
            nc.scalar.activation(sT[:], sembT_ps, AF.Identity, bias=bias_col[:])
            ssq = sb.tile([D, S], F32, bufs=1)
            nc.scalar.activation(ssq[:], sembT_ps, AF.Square, bias=bias_col[:])
            ns_ps = ps_t.tile([S, 1], F32, tag="tp")
            nc.tensor.matmul(ns_ps[:], lhsT=ssq[:], rhs=ones_col[:], start=True, stop=True)
            nsc = sb.tile([S, 1], F32, bufs=1)
            nc.vector.tensor_copy(nsc[:], ns_ps[:])
            nsi = sb.tile([S, 1], F32, bufs=1)
            nc.vector.reciprocal(nsi[:], nsc[:])
            cs = sb.tile([S, 1], F32, bufs=1)
            nc.scalar.sqrt(cs[:], nsi[:])  # 1/||se||
            semb_ps = ps_t.tile([S, D], F32, tag="tp")
            nc.tensor.transpose(semb_ps[:], sT[:], ident[:D, :D])
            sn = sb.tile([S, D], F32, bufs=1)
            nc.scalar.mul(sn[:], semb_ps[:], cs[:])
            snT_ps = ps_t.tile([D, S], F32, tag="tp")
            nc.tensor.transpose(snT_ps[:], sn[:], ident[:S, :S])
            snT = sb.tile([D, S], F32, bufs=1)
            nc.vector.tensor_copy(snT[:], snT_ps[:])

            # ---- query tail, phased so the ACT function-table only switches
            # once (SQRT bank -> EXP bank); alternating per-j cost 1.28us per
            # ACT_TABLE_LOAD on the critical path. ----
            qT = sb.tile([D, QC], F32, bufs=1)
            nc.scalar.activation(qT[:], embq_ps, AF.Identity, bias=bias_col[:])
            sqT = sb.tile([D, QC], F32, bufs=1)
            nc.scalar.activation(sqT[:], embq_ps, AF.Square, bias=bias_col[:])

            NJ = QC // 128
            aq, sims_ps, etile, denom, rden = [], [], [], [], []
            # phase B: query norms (all SQRTs together)
            for j in range(NJ):
                qs = slice(j * 128, (j + 1) * 128)
                nq_ps = ps_t.tile([128, 1], F32, tag="tp")
                nc.tensor.matmul(
                    nq_ps[:], lhsT=sqT[:, qs], rhs=ones_col[:], start=True, stop=True
                )
                nqi = sb.tile([128, 1], F32, tag="nqi")
                nc.vector.reciprocal(nqi[:], nq_ps[:])
                a = sb.tile([128, 1], F32, tag="aq")
                nc.scalar.sqrt(a[:], nqi[:])  # 1/||qe||
                aq.append(a)
            # phase C: sims + exp (all EXPs together; one table switch)
            for j in range(NJ):
                qs = slice(j * 128, (j + 1) * 128)
                sp = ps_t.tile([128, S], F32, tag="tp")
                nc.tensor.matmul(sp[:], lhsT=qT[:, qs], rhs=snT[:], start=True, stop=True)
                sims_ps.append(sp)
                # exp(sims * 1/||qe||) directly: cosines are in [-1, 1] so the
                # unstabilized softmax cannot overflow.
                e = sb.tile([128, S], F32, tag="etile")
                dn = sb.tile([128, 1], F32, tag="denom")
                nc.scalar.activation(e[:], sp[:], AF.Exp, scale=aq[j][:], accum_out=dn[:])
                etile.append(e)
                denom.append(dn)
            # phase D: denominators, transposed one_hot segment-sum, output
            for j in range(NJ):
                r = sb.tile([128, 1], F32, tag="rden")
                nc.vector.reciprocal(r[:], denom[j][:])
                rden.append(r)
            eT_all = sb.tile([S, QC], F32, bufs=1)
            for j in range(NJ):
                eT_ps = ps_t.tile([S, 128], F32, tag="tp")
                nc.tensor.transpose(eT_ps[:], etile[j][:], ident[:])
                nc.vector.tensor_copy(eT_all[:, j * 128 : (j + 1) * 128], eT_ps[:])
            numT_ps = ps_t.tile([NWAY, QC], F32, tag="numT_ps", bufs=1)
            nc.tensor.matmul(
                numT_ps[:], lhsT=oh_sb[:], rhs=eT_all[:], start=True, stop=True
            )
            numT = sb.tile([NWAY, QC], F32, bufs=1)
            nc.vector.tensor_copy(numT[:], numT_ps[:])
            for j in range(NJ):
                qs = slice(j * 128, (j + 1) * 128)
                num_ps = ps_t.tile([128, NWAY], F32, tag="tp")
                nc.tensor.transpose(num_ps[:], numT[:, qs], ident[:NWAY, :NWAY])
                lg = sb.tile([128, NWAY], F32, tag="lg")
                nc.vector.tensor_scalar_mul(lg[:], num_ps[:], rden[j][:])
                nc.sync.dma_start(out[qs, :], lg[:])
    return nc


def legalize_single_wait(nc):
    """Split multi-wait instructions: this walrus build allows at most ONE
    sync wait per instruction, so hoist extra waits onto same-engine NoOps
    inserted immediately before the instruction (identical semantics: the
    engine executes its queue in order)."""
    import bass_rust

    ctr = 0
    nsplit = 0
    for f in nc.m.functions:
        for bb in f.blocks:
            il = bb.instructions
            i = 0
            while i < len(il):
                ins = il[i]
                si = getattr(ins, "sync_info", None)
                if si is not None and len(si.on_wait) > 1:
                    waits = list(si.on_wait)
                    for w in waits[:-1]:
                        nop = bass_rust.InstNoOp(name=f"W-split-{ctr}")
                        ctr += 1
                        nop.engine = ins.engine
                        nop.sync_info = bass_rust.SyncInfo(on_wait=[w], on_update=[])
                        il.insert(i, nop)
                        i += 1
                    ins.sync_info = bass_rust.SyncInfo(
                        on_wait=[waits[-1]], on_update=list(si.on_update)
                    )
                    nsplit += 1
                i += 1
    # verify the rewrite took (bb.instructions must be a live list)
    remaining = sum(
        1
        for f in nc.m.functions
        for bb in f.blocks
        for ins in bb.instructions
        if getattr(ins, "sync_info", None) is not None
        and len(ins.sync_info.on_wait) > 1
    )
    assert remaining == 0, f"legalize_single_wait: {remaining} multi-wait instrs left"
    return nc


def pack_combined(W_, Xs_, Xq_core, t_tiles=T):
    """Build the combined [128, t_tiles*TW] bf16 buffer.

    Per K-tile t (contraction rows t*128..t*128+127):
      cols [0:64)    = W rows           (w_t[p, j]  = W[t*128+p, j])
      cols [64:164)  = Xs^T rows        (xs_t[p, j] = Xs[j, t*128+p])
      cols [164:676) = Xq_core^T rows   (xq_t[p, j] = Xq[j, t*128+p])
    """
    kext = t_tiles * KP
    A = np.zeros((KP, t_tiles, TW), dtype=BF16_NP)

    Wp = np.zeros((kext, D), dtype=BF16_NP)
    Wp[: W_.shape[0]] = W_.astype(BF16_NP)
    A[:, :, :D] = Wp.reshape(t_tiles, KP, D).transpose(1, 0, 2)

    def rows_pack(X, n):
        Xp = np.zeros((n, kext), dtype=BF16_NP)
        Xp[:, : X.shape[1]] = X.astype(BF16_NP)
        return Xp.reshape(n, t_tiles, KP).transpose(2, 1, 0)  # [128, t, n]

    A[:, :, D : D + S] = rows_pack(Xs_, S)
    A[:, :, D + S :] = rows_pack(Xq_core, QC)
    return np.ascontiguousarray(A.reshape(KP, t_tiles * TW))


def make_in_maps(support_images, support_labels, query_images, backbone_w, backbone_b):
    Xq = np.asarray(query_images, dtype=np.float32)
    Xs = np.asarray(support_images, dtype=np.float32)
    W = np.asarray(backbone_w, dtype=np.float32)
    b = np.asarray(backbone_b, dtype=np.float32).reshape(D, 1)
    labels = np.asarray(support_labels).astype(np.int64).reshape(S)
    onehot = np.zeros((S, NWAY), np.float32)
    onehot[np.arange(S), labels] = 1.0

    common = {
        "bias": b,
        "onehot": onehot,
        "identity": np.eye(KP, dtype=np.float32),
        "ones": np.ones((D, 1), np.float32),
    }
    in_maps = []
    for c in range(NCORES):
        data_c = pack_combined(W, Xs, Xq[c * QC : (c + 1) * QC])
        in_maps.append({"data": data_c, **common})
    return in_maps


def run(in_maps, trace=False, **kw):
    nc = build_bass()
    legalize_single_wait(nc)
    return run_bass_kernel_spmd(nc, in_maps, list(range(NCORES)), trace=trace, **kw)


def kernel(
    support_images,
    support_labels,
    query_images,
    n_way,
    k_shot,
    backbone_w,
    backbone_b,
):
    assert int(n_way) == NWAY
    in_maps = make_in_maps(
        support_images, support_labels, query_images, backbone_w, backbone_b
    )
    res = run(in_maps, trace=False)
    return np.concatenate(
        [np.asarray(res.results[c]["out"]) for c in range(NCORES)], axis=0
    )
